# revision 2
# baseline (speedup 1.0000x reference)
"""Trainium2 Bass kernel for nn_LinearGaussianQ (folded-native bf16).

Reference: T=256-step linear-Gaussian smoother scan with a growing stack of
quadratic forms.  Reformulated (validated to ~2e-5 rel in f64):

  * parameter-only work (Kalman covariance pipeline, log-dets, trace series)
    is precomputed on host in f64 and folded into one constant, exactly like
    the reference precomputes its parameter inverses;
  * everything touching `observations` runs on device in bf16 (tolerance is
    2e-2; bf16 end-to-end error ~5e-5): the forward filter-mean recursion and
    the backward e-recursion become chunked matrix scans in a folded layout
    [128, C] with time t = 8c + l (partition-block l, column c) — one local
    block-triangular-Toeplitz [128x128] matmul plus a 1-chunk-band
    block-Toeplitz correction (state decay ||F^9|| ~ 2e-3, ||F^17|| ~ 3e-6
    makes the window truncation negligible);
  * the first TSTAR=8 steps use exact time-varying maps composed on host
    (F_t/B_t converge to the steady-state maps at ~1e-6 by t=8);
  * all quadratic-form data terms are evaluated folded on all 128 partitions
    and reduced on-device to a single scalar.

Sharding: the recursion is strictly sequential with tiny state (batch=1), so
the data-parallel hint degenerates; all 8 cores run the identical replicated
program and core 0's scalar is returned.
"""
import numpy as np
import ml_dtypes

T = 256
DZ = 16
TS = 8
LOG2PI = float(np.log(2.0 * np.pi))
F32 = np.float32
BF16 = ml_dtypes.bfloat16

_PROGRAM_CACHE = {}


def host_prep_v5(inputs):
    o = {k: np.asarray(v, np.float64) for k, v in inputs.items()}
    dz = DZ
    I = np.eye(dz)

    def cterm(dim, det):
        return -0.5 * (dim * LOG2PI + np.log(det))

    p_tr_prec = np.linalg.inv(o["p_trans_cov"])
    p_tr_det = np.linalg.det(o["p_trans_cov"])
    p_em_prec = np.linalg.inv(o["p_em_cov"])
    p_em_det = np.linalg.det(o["p_em_cov"])
    q_tr_prec = np.linalg.inv(o["q_trans_cov"])
    Om_obs = -0.5 * p_em_prec
    Om_tr = -0.5 * p_tr_prec
    Om0 = -0.5 * np.linalg.inv(o["p_prior_cov"])
    qW, qb = o["q_trans_w"], o["q_trans_b"]
    qC = o["q_trans_cov"]
    H, h, Rm = o["q_em_w"], o["q_em_b"], o["q_em_cov"]
    pW, pb = o["p_trans_w"], o["p_trans_b"]
    pH, ph = o["p_em_w"], o["p_em_b"]
    cm = qW.T @ q_tr_prec
    Phi = cm @ qW
    Cobs = pH.T @ Om_obs @ pH
    Ctr = -0.5 * pW.T @ p_tr_prec @ pW
    c1 = (cterm(dz, p_em_det) + cterm(dz, p_tr_det) + 0.5 * dz
          + 0.5 * dz * LOG2PI)

    def kgain(P_pred):
        S = H @ P_pred @ H.T + Rm
        Kg = P_pred @ H.T @ np.linalg.inv(S)
        return Kg, (I - Kg @ H) @ P_pred

    Kg0, P0 = kgain(o["q_prior_cov"])
    Pf = [P0]
    Kgs = [Kg0]
    Bs = [None]
    bcovs = [None]
    Ams = [None]
    abts = [None]
    for t in range(1, T):
        Pprev = Pf[-1]
        P_prec = np.linalg.inv(Pprev)
        bcov = np.linalg.inv(Phi + P_prec)
        Bs.append(bcov @ cm)
        bcovs.append(bcov)
        Ams.append(np.linalg.inv(I + Pprev @ Phi))
        abts.append(-(bcov @ cm @ qb))
        Kg, Pnew = kgain(qW @ Pprev @ qW.T + qC)
        Pf.append(Pnew)
        Kgs.append(Kg)

    const = cterm(dz, np.linalg.det(o["p_prior_cov"])) + cterm(dz, p_em_det)
    M = Om0.copy()
    for t in range(1, T):
        bcov = bcovs[t]
        const += np.trace((M + Cobs + Ctr) @ bcov)
        const += 0.5 * np.log(np.linalg.det(bcov)) + c1
        B = Bs[t]
        M = B.T @ (M + Cobs) @ B + (pW @ B - I).T @ Om_tr @ (pW @ B - I)
    const -= cterm(dz, np.linalg.det(Pf[-1]))

    P_ss = Pf[-1]
    tr = 0.0
    Rt = {T - 1: np.eye(dz)}
    for t in range(T - 2, TS - 1, -1):
        Rt[t] = Bs[t + 1] @ Rt[t + 1]
    for t in range(1, T):
        Rm1 = Rt.get(t - 1)
        Rcur = Rt.get(t)
        if Rm1 is None or Rcur is None:
            continue
        G = pH @ Rm1
        tr += np.einsum('ij,jl,lm,mi->', Om_obs, G, P_ss, G)
        Ae = pW @ Rm1 - Rcur
        tr += np.einsum('ij,jl,lm,mi->', Om_tr, Ae, P_ss, Ae)
    tr_p = np.trace(Om_obs @ pH @ P_ss @ pH)
    const_host = const + tr + tr_p + 0.5 * dz

    Fs = [None] + [(I - Kgs[t] @ H) @ qW for t in range(1, T)]
    c0s = [None] + [(I - Kgs[t] @ H) @ qb - Kgs[t] @ h for t in range(1, T)]

    A0 = Kg0
    b0 = (I - Kg0 @ H) @ o["q_prior_mean"] - Kg0 @ h
    Gp = [{0: A0}]
    ccp = [b0]
    for t in range(1, TS + 1):
        g = {s: Fs[t] @ mm for s, mm in Gp[t - 1].items()}
        g[t] = Kgs[t].copy()
        Gp.append(g)
        ccp.append(Fs[t] @ ccp[t - 1] + c0s[t])

    W8 = np.zeros((9 * 16, 16))
    for s, g in Gp[TS].items():
        W8[s * 16:(s + 1) * 16, :] = g.T
    m8b = ccp[TS]

    WA = np.zeros((9 * 16, 8 * 16))
    ab_bias = np.zeros(8 * 16)
    for t in range(1, TS + 1):
        for s, g in Gp[t - 1].items():
            AG = Ams[t] @ g
            WA[s * 16:(s + 1) * 16, (t - 1) * 16:t * 16] = AG.T
        ac = Ams[t] @ ccp[t - 1] + abts[t]
        ab_bias[(t - 1) * 16:t * 16] = ac

    Ec = {TS: I.copy()}
    Dc = {TS: {}}
    for s in range(TS - 1, -1, -1):
        Ec[s] = Bs[s + 1] @ Ec[s + 1]
        Dc[s] = {u: Bs[s + 1] @ g for u, g in Dc[s + 1].items()}
        Dc[s][s + 1] = I.copy()
    WE = np.zeros((8 * 16, 8 * 16))
    for s in range(TS):
        for u, g in Dc[s].items():
            WE[(u - 1) * 16:u * 16, s * 16:(s + 1) * 16] = g.T

    return dict(
        const_host=const_host,
        F=Fs[-1], B=Bs[-1], Kg=Kgs[-1], c0=c0s[-1],
        Am=Ams[-1], ab=abts[-1],
        pH=pH, ph=ph, pW=pW, pb=pb,
        Om_obs=Om_obs, Om_tr=Om_tr, Om0=Om0, pm=o["p_prior_mean"],
        W8=W8, m8b=m8b, WA=WA, ab_bias=ab_bias, WE=WE,
        Ec=[Ec[s] for s in range(TS)],
    )


def bd8(A):
    out = np.zeros((128, 128))
    for l in range(8):
        out[16 * l:16 * l + 16, 16 * l:16 * l + 16] = A
    return out


def tile8(v):
    return np.tile(np.asarray(v).reshape(-1), 8).reshape(128, 1)


def build_mats_v5(hp):
    F, B = hp["F"], hp["B"]
    m = {}
    Fp = [np.eye(DZ)]
    Bp = [np.eye(DZ)]
    for _ in range(26):
        Fp.append(Fp[-1] @ F)
        Bp.append(Bp[-1] @ B)
    SF = np.zeros((128, 128))
    SB = np.zeros((128, 128))
    for l in range(8):
        for i in range(8):
            if i <= l:
                SF[16 * l:16 * l + 16, 16 * i:16 * i + 16] = Fp[l - i]
            if i >= l:
                SB[16 * l:16 * l + 16, 16 * i:16 * i + 16] = Bp[i - l]
    m["SF"], m["SB"] = SF, SB
    for k in (1, 2):
        G = np.zeros((128, 128))
        Hk = np.zeros((128, 128))
        for l in range(8):
            for i in range(8):
                G[16 * l:16 * l + 16, 16 * i:16 * i + 16] = Fp[8 * k + l - i]
                Hk[16 * l:16 * l + 16, 16 * i:16 * i + 16] = Bp[8 * k + i - l]
        m[f"G{k}"] = G
        m[f"H{k}"] = Hk
    m["bdKg"] = bd8(hp["Kg"])
    m["bdAm"] = bd8(hp["Am"])
    m["bdpH"] = bd8(hp["pH"])
    SU = np.zeros((128, 128))
    for l in range(7):
        SU[16 * l:16 * l + 16, 16 * (l + 1):16 * (l + 1) + 16] = np.eye(16)
    m["bdpWmSU"] = bd8(hp["pW"]) - SU
    m["bdOmO"] = bd8(hp["Om_obs"])
    m["bdOmT"] = bd8(hp["Om_tr"])
    m["Om0"] = hp["Om0"]
    m["corrAm"] = np.eye(16) - hp["Am"]     # applied to block 7
    m["negpW"] = -hp["pW"]
    m["WEWA"] = hp["WA"] @ hp["WE"]          # [144, 128]
    m["webias"] = hp["WE"].T @ hp["ab_bias"]  # [128]
    m["W8"] = hp["W8"]
    m["W8c1mKg"] = hp["W8"][128:144] - hp["Kg"].T
    ECF = np.zeros((128, 16))
    for s in range(8):
        ECF[16 * s:16 * s + 16, :] = hp["Ec"][s]
    m["ECF"] = ECF
    m["m8b"] = hp["m8b"]
    m["c0f"] = tile8(hp["c0"])
    m["abf"] = tile8(hp["ab"])
    m["phf"] = tile8(hp["ph"])
    m["pbf"] = tile8(hp["pb"])
    m["pm"] = hp["pm"]
    m["const_host"] = hp["const_host"]
    return m



def _pack(m):
    def packcols(entries, prows=128):
        cols = []
        off = {}
        n = 0
        for name, arr, r0, rows in entries:
            a = np.zeros((prows, arr.shape[1]))
            a[r0:r0 + rows] = arr[:rows]
            off[name] = n
            n += arr.shape[1]
            cols.append(a)
        return np.concatenate(cols, 1), off

    corrAmlo = np.zeros((128, 128))
    corrAmlo[112:128, 112:128] = m["corrAm"]
    negpWlo = np.zeros((128, 128))
    negpWlo[112:128, 112:128] = m["negpW"]
    negCN = np.zeros((128, 16))
    negCN[112:128] = -np.eye(16)
    om0pad = np.zeros((16, 128))
    om0pad[:, 0:16] = m["Om0"].T
    cid, offI = packcols([("ident32", np.eye(32), 0, 32)], prows=32)
    cT1a, off1a = packcols([
        ("W8c0", m["W8"][0:128], 0, 128),
        ("bdKg", m["bdKg"].T, 0, 128),
        ("W8c1mKg", np.vstack([m["W8c1mKg"], np.zeros((112, 16))]), 0, 16),
    ])
    cT1b, off1b = packcols([
        ("SF", m["SF"].T, 0, 128),
        ("G1", m["G1"].T, 0, 128),
    ])
    cT2a, off2a = packcols([
        ("bdAm", m["bdAm"].T, 0, 128),
        ("corrAmlo", corrAmlo.T[64:128], 64, 64),
    ])
    cT2b, off2b = packcols([
        ("SB", m["SB"].T, 0, 128),
        ("H1", m["H1"].T, 0, 128),
        ("ww0", m["WEWA"][0:128], 0, 128),
    ])
    cT3, off3 = packcols([
        ("bdpH", m["bdpH"].T, 0, 128),
        ("bdpWmSU", m["bdpWmSU"].T, 0, 128),
        ("negpWlo", negpWlo.T[64:128], 64, 64),
        ("bdOmO", m["bdOmO"].T, 0, 128),
        ("bdOmT", m["bdOmT"].T, 0, 128),
    ])
    c16t, off16 = packcols([
        ("negCN16", negCN.T, 0, 16),
        ("Om0pad16", om0pad, 0, 16),
        ("ww1", m["WEWA"][128:144], 0, 16),
        ("ECF8", m["ECF"].T, 0, 16),
        ("pm16", m["pm"].reshape(16, 1), 0, 16),
        ("webias_r", m["webias"].reshape(1, 128), 0, 1),
        ("onesb", np.ones((1, 1)), 0, 1),
    ], prows=16)
    cFm = np.zeros((128, 10))
    cFm[0, 0] = m["const_host"]
    cFm[:, 1] = 1.0
    cFm[:, 2:3] = m["c0f"]
    cFm[:, 3:4] = m["abf"]
    cFm[:, 4:5] = m["phf"]
    cFm[:, 5:6] = m["pbf"]
    cFm[:, 6:7] = m["abf"]
    cFm[112:128, 6] = 0.0
    cFm[:, 7:8] = m["pbf"]
    cFm[112:128, 7] = 0.0
    cFm[:, 8:9] = m["c0f"]
    cFm[0:16, 8] = m["m8b"]
    cFm[:, 9] = m["webias"].reshape(128)
    offF = {"chost": 0, "ones": 1, "c0f": 2, "abf": 3, "phf": 4,
            "pbf": 5, "abf7z": 6, "pbf7z": 7, "mixbias": 8, "webias": 9}
    return {
        "cid": (cid.astype(BF16), offI),
        "cT1a": (cT1a.astype(BF16), off1a),
        "cT1b": (cT1b.astype(BF16), off1b),
        "cT2a": (cT2a.astype(BF16), off2a),
        "cT2b": (cT2b.astype(BF16), off2b),
        "cT3": (cT3.astype(BF16), off3),
        "c16t": (c16t.astype(BF16), off16),
        "cF": (cFm.astype(F32), offF),
    }


def _build_program(shapes, offs):
    import concourse.bacc as bacc
    import concourse.mybir as mybir
    from concourse import tile

    f32 = mybir.dt.float32
    bf16 = mybir.dt.bfloat16
    ALU = mybir.AluOpType
    ACT = mybir.ActivationFunctionType
    nc = bacc.Bacc("TRN2", target_bir_lowering=False, debug=False)
    obs_d = nc.declare_dram_parameter("obs", [32, 128], bf16, isOutput=False)
    cdr = {}
    for nm in ("cid", "cT1a", "cT1b", "cT2a", "cT2b", "cT3", "c16t"):
        rows = {"cid": 32, "c16t": 16}.get(nm, 128)
        cdr[nm] = nc.declare_dram_parameter(nm, [rows, shapes[nm][1]], bf16,
                                            isOutput=False)
    cF_d = nc.declare_dram_parameter("cF", list(shapes["cF"]), f32,
                                     isOutput=False)
    out_d = nc.declare_dram_parameter("out", [1, 1], f32, isOutput=True)

    oI, o1a, o1b, o2a, o2b, o3, o16, oF = (offs[n] for n in
        ("cid", "cT1a", "cT1b", "cT2a", "cT2b", "cT3", "c16t", "cF"))

    with tile.TileContext(nc) as tc:
        with (
            tc.tile_pool(name="const", bufs=1) as cpool,
            tc.tile_pool(name="sb", bufs=1) as sb,
            tc.tile_pool(name="ps", bufs=2, space="PSUM") as ps,
        ):
            tI = cpool.tile([32, shapes["cid"][1]], bf16)
            t1a = cpool.tile([128, shapes["cT1a"][1]], bf16)
            t1b = cpool.tile([128, shapes["cT1b"][1]], bf16)
            t2a = cpool.tile([128, shapes["cT2a"][1]], bf16)
            t2b = cpool.tile([128, shapes["cT2b"][1]], bf16)
            t3 = cpool.tile([128, shapes["cT3"][1]], bf16)
            t16 = cpool.tile([16, shapes["c16t"][1]], bf16)
            tF = cpool.tile([128, shapes["cF"][1]], f32)
            ob = sb.tile([32, 128], bf16)
            nc.sync.dma_start(ob[:], obs_d[:])
            nc.sync.dma_start(tI[:], cdr["cid"][:])
            nc.sync.dma_start(t1a[:], cdr["cT1a"][:])
            nc.scalar.dma_start(t1b[:], cdr["cT1b"][:])
            nc.gpsimd.dma_start(tF[:], cF_d[:])
            nc.gpsimd.dma_start(t16[:], cdr["c16t"][:])
            nc.sync.dma_start(t2a[:], cdr["cT2a"][:])
            nc.scalar.dma_start(t2b[:], cdr["cT2b"][:])
            h3 = shapes["cT3"][1] // 2
            nc.gpsimd.dma_start(t3[:, 0:h3], cdr["cT3"][:, 0:h3])
            nc.scalar.dma_start(t3[:, h3:], cdr["cT3"][:, h3:])

            # ---- Yf = transpose(ob) ----
            psY = ps.tile([128, 32], bf16, tag="pA")
            nc.tensor.transpose(psY[:], ob[:], tI[0:32, 0:32])
            Yf = sb.tile([128, 32], bf16)
            nc.vector.tensor_copy(Yf[:], psY[:])

            def K1(name, w=128):
                t, o = (t1a, o1a) if name in o1a else (t1b, o1b)
                return t[0:128, o[name]:o[name] + w]

            def K2(name, w=128):
                t, o = (t2a, o2a) if name in o2a else (t2b, o2b)
                return t[0:128, o[name]:o[name] + w]

            def K2lo(name, w=128):
                return t2a[64:128, o2a[name]:o2a[name] + w]

            def K3(name, w=128):
                return t3[0:128, o3[name]:o3[name] + w]

            def K3lo(name, w=128):
                return t3[64:128, o3[name]:o3[name] + w]

            def K16(name, w=128):
                return t16[0:16, o16[name]:o16[name] + w]

            def KF(name):
                return tF[:, oF[name]:oF[name] + 1]

            # ---- psJ (j-grid, 31 cols); col0 block0 += W8 path ----
            psJ = ps.tile([128, 31], f32, tag="pB")
            nc.tensor.matmul(psJ[:], K1("bdKg"), Yf[:, 1:32],
                             start=True, stop=False)
            nc.tensor.matmul(psJ[0:16, 0:1], K1("W8c0", 16), Yf[:, 0:1],
                             start=False, stop=False)
            nc.tensor.matmul(psJ[0:16, 0:1],
                             t1a[0:16, o1a["W8c1mKg"]:o1a["W8c1mKg"] + 16],
                             Yf[0:16, 1:2], start=False, stop=True)
            X = sb.tile([128, 31], bf16)
            nc.vector.tensor_scalar_add(X[:, 1:31], psJ[:, 1:31], KF("c0f"))
            nc.vector.tensor_add(X[:, 0:1], psJ[:, 0:1], KF("mixbias"))

            # ---- forward conv scan (one accumulation group) ----
            psL = ps.tile([128, 31], f32, tag="pA")
            nc.tensor.matmul(psL[:], K1("SF"), X[:], start=True, stop=False)
            nc.tensor.matmul(psL[:, 1:31], K1("G1"), X[:, 0:30],
                             start=False, stop=True)
            Mf = sb.tile([128, 31], bf16)
            nc.vector.tensor_copy(Mf[:], psL[:])

            # ---- Ef[:,1:32] = bdAm @ Mf + abf (col31: corrAm, abf7z) ----
            Ef = sb.tile([128, 32], bf16)
            psA = ps.tile([128, 31], f32, tag="pB")
            nc.tensor.matmul(psA[:], K2("bdAm"), Mf[:], start=True,
                             stop=False)
            nc.tensor.matmul(psA[:, 30:31], K2lo("corrAmlo"),
                             Mf[64:128, 30:31], start=False, stop=True)
            nc.vector.tensor_scalar_add(Ef[:, 1:31], psA[:, 0:30], KF("abf"))
            nc.vector.tensor_add(Ef[:, 31:32], psA[:, 30:31], KF("abf7z"))

            # ---- backward conv scan ----
            psB = ps.tile([128, 31], f32, tag="pA")
            nc.tensor.matmul(psB[:], K2("SB"), Ef[:, 1:32],
                             start=True, stop=False)
            nc.tensor.matmul(psB[:, 0:30], K2("H1"), Ef[:, 2:32],
                             start=False, stop=True)
            nc.vector.tensor_copy(Ef[:, 1:32], psB[:])

            # ---- transient col 0: one accumulation group then copy ----
            psC = ps.tile([128, 1], f32, tag="pE")
            nc.tensor.matmul(psC[:], K2("ww0"), Yf[:, 0:1],
                             start=True, stop=False)
            nc.tensor.matmul(psC[:], K16("ww1"), Yf[0:16, 1:2],
                             start=False, stop=False)
            nc.tensor.matmul(psC[:], K16("webias_r")[0:1, :],
                             K16("onesb", 1)[0:1, :], start=False, stop=False)
            nc.tensor.matmul(psC[:], K16("ECF8"), Ef[0:16, 1:2],
                             start=False, stop=True)
            nc.vector.tensor_copy(Ef[:, 0:1], psC[:])

            # ---- UOE ----
            UOE = sb.tile([128, 65], bf16)
            nc.gpsimd.memset(UOE[:, 64:65], 0.0)
            nc.vector.tensor_sub(UOE[0:16, 64:65], Ef[0:16, 0:1],
                                 K16("pm16", 1))
            psUO = ps.tile([128, 32], f32, tag="pB")
            nc.tensor.matmul(psUO[:], K3("bdpH"), Ef[:])
            nc.vector.scalar_tensor_tensor(UOE[:, 0:32], psUO[:], KF("phf"),
                                           Yf[:], op0=ALU.add,
                                           op1=ALU.subtract)
            psUE = ps.tile([128, 32], f32, tag="pA")
            nc.tensor.matmul(psUE[:], K3("bdpWmSU"), Ef[:],
                             start=True, stop=False)
            nc.tensor.matmul(psUE[:, 0:31], K16("negCN16"), Ef[0:16, 1:32],
                             start=False, stop=False)
            nc.tensor.matmul(psUE[:, 31:32], K3lo("negpWlo"),
                             Ef[64:128, 31:32], start=False, stop=True)
            nc.scalar.activation(UOE[:, 32:63], psUE[:, 0:31], ACT.Identity,
                                 bias=KF("pbf"))
            nc.vector.tensor_add(UOE[:, 63:64], psUE[:, 31:32],
                                  KF("pbf7z"))

            # ---- Z ; fused quad-reduce ----
            psZ = ps.tile([128, 65], f32, tag="pE")
            nc.tensor.matmul(psZ[:, 0:32], K3("bdOmO"), UOE[:, 0:32])
            nc.tensor.matmul(psZ[:, 32:64], K3("bdOmT"), UOE[:, 32:64])
            nc.tensor.matmul(psZ[:, 64:65], K16("Om0pad16"),
                             UOE[0:16, 64:65])
            P = sb.tile([128, 65], f32)
            acc = sb.tile([128, 1], f32)
            nc.vector.scalar_tensor_tensor(P[:], psZ[:], 1.0, UOE[:],
                                           op0=ALU.mult, op1=ALU.mult,
                                           accum_out=acc[:])
            psT = ps.tile([1, 1], f32, tag="pB")
            nc.tensor.matmul(psT[:], KF("ones"), acc[:],
                             start=True, stop=False)
            nc.tensor.matmul(psT[:], KF("chost"), KF("ones"),
                             start=False, stop=True)
            res = sb.tile([1, 1], f32)
            nc.vector.tensor_copy(res[:], psT[:])
            nc.sync.dma_start(out_d[:], res[:])

    nc.finalize()
    return nc


def _get_program(tabs):
    shapes = {k: v[0].shape for k, v in tabs.items()}
    offs = {k: v[1] for k, v in tabs.items()}
    key = tuple(sorted((k, s) for k, s in shapes.items()))
    if key not in _PROGRAM_CACHE:
        _PROGRAM_CACHE[key] = _build_program(shapes, offs)
    return _PROGRAM_CACHE[key]


def kernel(**inputs):
    from concourse.bass_utils import run_bass_kernel_spmd

    hp = host_prep_v5(inputs)
    m = build_mats_v5(hp)
    tabs = _pack(m)
    obs = np.asarray(inputs["observations"], F32).astype(BF16).reshape(32, 128)
    nc = _get_program(tabs)
    in_map = {"obs": obs}
    for k, (arr, _) in tabs.items():
        in_map[k] = arr
    res = run_bass_kernel_spmd(nc, [dict(in_map) for _ in range(8)],
                               list(range(8)))
    out = res.results[0]["out"]
    return np.asarray(out, dtype=np.float32).reshape(())


# revision 3
# speedup vs baseline: 1.0273x; 1.0273x over previous
"""Trainium2 Bass kernel for nn_LinearGaussianQ (folded-native bf16).

Reference: T=256-step linear-Gaussian smoother scan with a growing stack of
quadratic forms.  Reformulated (validated to ~2e-5 rel in f64):

  * parameter-only work (Kalman covariance pipeline, log-dets, trace series)
    is precomputed on host in f64 and folded into one constant, exactly like
    the reference precomputes its parameter inverses;
  * everything touching `observations` runs on device in bf16 (tolerance is
    2e-2; bf16 end-to-end error ~5e-5): the forward filter-mean recursion and
    the backward e-recursion become chunked matrix scans in a folded layout
    [128, C] with time t = 8c + l (partition-block l, column c) — one local
    block-triangular-Toeplitz [128x128] matmul plus a 1-chunk-band
    block-Toeplitz correction (state decay ||F^9|| ~ 2e-3, ||F^17|| ~ 3e-6
    makes the window truncation negligible);
  * the first TS=8 steps use exact time-varying maps composed on host
    (F_t/B_t converge to the steady-state maps at ~1e-6 by t=8);
  * all quadratic-form data terms are evaluated folded on all 128 partitions
    and reduced on-device to a single scalar.

Sharding: the recursion is strictly sequential with tiny state (batch=1), so
the data-parallel hint degenerates; all 8 cores run the identical replicated
program and core 0's scalar is returned.
"""
import numpy as np
import ml_dtypes

T = 256
DZ = 16
TS = 8
LOG2PI = float(np.log(2.0 * np.pi))
F32 = np.float32
BF16 = ml_dtypes.bfloat16

_PROGRAM_CACHE = {}


def host_prep_v5(inputs):
    o = {k: np.asarray(v, np.float64) for k, v in inputs.items()}
    dz = DZ
    I = np.eye(dz)

    def cterm(dim, det):
        return -0.5 * (dim * LOG2PI + np.log(det))

    p_tr_prec = np.linalg.inv(o["p_trans_cov"])
    p_tr_det = np.linalg.det(o["p_trans_cov"])
    p_em_prec = np.linalg.inv(o["p_em_cov"])
    p_em_det = np.linalg.det(o["p_em_cov"])
    q_tr_prec = np.linalg.inv(o["q_trans_cov"])
    Om_obs = -0.5 * p_em_prec
    Om_tr = -0.5 * p_tr_prec
    Om0 = -0.5 * np.linalg.inv(o["p_prior_cov"])
    qW, qb = o["q_trans_w"], o["q_trans_b"]
    qC = o["q_trans_cov"]
    H, h, Rm = o["q_em_w"], o["q_em_b"], o["q_em_cov"]
    pW, pb = o["p_trans_w"], o["p_trans_b"]
    pH, ph = o["p_em_w"], o["p_em_b"]
    cm = qW.T @ q_tr_prec
    Phi = cm @ qW
    Cobs = pH.T @ Om_obs @ pH
    Ctr = -0.5 * pW.T @ p_tr_prec @ pW
    c1 = (cterm(dz, p_em_det) + cterm(dz, p_tr_det) + 0.5 * dz
          + 0.5 * dz * LOG2PI)

    def kgain(P_pred):
        S = H @ P_pred @ H.T + Rm
        Kg = P_pred @ H.T @ np.linalg.inv(S)
        return Kg, (I - Kg @ H) @ P_pred

    Kg0, P0 = kgain(o["q_prior_cov"])
    Pf = [P0]
    Kgs = [Kg0]
    Bs = [None]
    bcovs = [None]
    Ams = [None]
    abts = [None]
    for t in range(1, T):
        Pprev = Pf[-1]
        P_prec = np.linalg.inv(Pprev)
        bcov = np.linalg.inv(Phi + P_prec)
        Bs.append(bcov @ cm)
        bcovs.append(bcov)
        Ams.append(np.linalg.inv(I + Pprev @ Phi))
        abts.append(-(bcov @ cm @ qb))
        Kg, Pnew = kgain(qW @ Pprev @ qW.T + qC)
        Pf.append(Pnew)
        Kgs.append(Kg)

    const = cterm(dz, np.linalg.det(o["p_prior_cov"])) + cterm(dz, p_em_det)
    M = Om0.copy()
    for t in range(1, T):
        bcov = bcovs[t]
        const += np.trace((M + Cobs + Ctr) @ bcov)
        const += 0.5 * np.log(np.linalg.det(bcov)) + c1
        B = Bs[t]
        M = B.T @ (M + Cobs) @ B + (pW @ B - I).T @ Om_tr @ (pW @ B - I)
    const -= cterm(dz, np.linalg.det(Pf[-1]))

    P_ss = Pf[-1]
    tr = 0.0
    Rt = {T - 1: np.eye(dz)}
    for t in range(T - 2, TS - 1, -1):
        Rt[t] = Bs[t + 1] @ Rt[t + 1]
    for t in range(1, T):
        Rm1 = Rt.get(t - 1)
        Rcur = Rt.get(t)
        if Rm1 is None or Rcur is None:
            continue
        G = pH @ Rm1
        tr += np.einsum('ij,jl,lm,mi->', Om_obs, G, P_ss, G)
        Ae = pW @ Rm1 - Rcur
        tr += np.einsum('ij,jl,lm,mi->', Om_tr, Ae, P_ss, Ae)
    tr_p = np.trace(Om_obs @ pH @ P_ss @ pH)
    const_host = const + tr + tr_p + 0.5 * dz

    Fs = [None] + [(I - Kgs[t] @ H) @ qW for t in range(1, T)]
    c0s = [None] + [(I - Kgs[t] @ H) @ qb - Kgs[t] @ h for t in range(1, T)]

    A0 = Kg0
    b0 = (I - Kg0 @ H) @ o["q_prior_mean"] - Kg0 @ h
    Gp = [{0: A0}]
    ccp = [b0]
    for t in range(1, TS + 1):
        g = {s: Fs[t] @ mm for s, mm in Gp[t - 1].items()}
        g[t] = Kgs[t].copy()
        Gp.append(g)
        ccp.append(Fs[t] @ ccp[t - 1] + c0s[t])

    W8 = np.zeros((9 * 16, 16))
    for s, g in Gp[TS].items():
        W8[s * 16:(s + 1) * 16, :] = g.T
    m8b = ccp[TS]

    WA = np.zeros((9 * 16, 8 * 16))
    ab_bias = np.zeros(8 * 16)
    for t in range(1, TS + 1):
        for s, g in Gp[t - 1].items():
            AG = Ams[t] @ g
            WA[s * 16:(s + 1) * 16, (t - 1) * 16:t * 16] = AG.T
        ac = Ams[t] @ ccp[t - 1] + abts[t]
        ab_bias[(t - 1) * 16:t * 16] = ac

    Ec = {TS: I.copy()}
    Dc = {TS: {}}
    for s in range(TS - 1, -1, -1):
        Ec[s] = Bs[s + 1] @ Ec[s + 1]
        Dc[s] = {u: Bs[s + 1] @ g for u, g in Dc[s + 1].items()}
        Dc[s][s + 1] = I.copy()
    WE = np.zeros((8 * 16, 8 * 16))
    for s in range(TS):
        for u, g in Dc[s].items():
            WE[(u - 1) * 16:u * 16, s * 16:(s + 1) * 16] = g.T

    return dict(
        const_host=const_host,
        F=Fs[-1], B=Bs[-1], Kg=Kgs[-1], c0=c0s[-1],
        Am=Ams[-1], ab=abts[-1],
        pH=pH, ph=ph, pW=pW, pb=pb,
        Om_obs=Om_obs, Om_tr=Om_tr, Om0=Om0, pm=o["p_prior_mean"],
        W8=W8, m8b=m8b, WA=WA, ab_bias=ab_bias, WE=WE,
        Ec=[Ec[s] for s in range(TS)],
    )


def bd8(A):
    out = np.zeros((128, 128))
    for l in range(8):
        out[16 * l:16 * l + 16, 16 * l:16 * l + 16] = A
    return out


def tile8(v):
    return np.tile(np.asarray(v).reshape(-1), 8).reshape(128, 1)


def build_mats_v5(hp):
    F, B = hp["F"], hp["B"]
    m = {}
    Fp = [np.eye(DZ)]
    Bp = [np.eye(DZ)]
    for _ in range(26):
        Fp.append(Fp[-1] @ F)
        Bp.append(Bp[-1] @ B)
    SF = np.zeros((128, 128))
    SB = np.zeros((128, 128))
    for l in range(8):
        for i in range(8):
            if i <= l:
                SF[16 * l:16 * l + 16, 16 * i:16 * i + 16] = Fp[l - i]
            if i >= l:
                SB[16 * l:16 * l + 16, 16 * i:16 * i + 16] = Bp[i - l]
    m["SF"], m["SB"] = SF, SB
    for k in (1, 2):
        G = np.zeros((128, 128))
        Hk = np.zeros((128, 128))
        for l in range(8):
            for i in range(8):
                G[16 * l:16 * l + 16, 16 * i:16 * i + 16] = Fp[8 * k + l - i]
                Hk[16 * l:16 * l + 16, 16 * i:16 * i + 16] = Bp[8 * k + i - l]
        m[f"G{k}"] = G
        m[f"H{k}"] = Hk
    m["bdKg"] = bd8(hp["Kg"])
    m["bdAm"] = bd8(hp["Am"])
    m["bdpH"] = bd8(hp["pH"])
    SU = np.zeros((128, 128))
    for l in range(7):
        SU[16 * l:16 * l + 16, 16 * (l + 1):16 * (l + 1) + 16] = np.eye(16)
    m["bdpWmSU"] = bd8(hp["pW"]) - SU
    m["bdOmO"] = bd8(hp["Om_obs"])
    m["bdOmT"] = bd8(hp["Om_tr"])
    m["Om0"] = hp["Om0"]
    m["corrAm"] = np.eye(16) - hp["Am"]     # applied to block 7
    m["negpW"] = -hp["pW"]
    m["WEWA"] = hp["WA"] @ hp["WE"]          # [144, 128]
    m["webias"] = hp["WE"].T @ hp["ab_bias"]  # [128]
    m["W8"] = hp["W8"]
    m["W8c1mKg"] = hp["W8"][128:144] - hp["Kg"].T
    ECF = np.zeros((128, 16))
    for s in range(8):
        ECF[16 * s:16 * s + 16, :] = hp["Ec"][s]
    m["ECF"] = ECF
    m["m8b"] = hp["m8b"]
    m["c0f"] = tile8(hp["c0"])
    m["abf"] = tile8(hp["ab"])
    m["phf"] = tile8(hp["ph"])
    m["pbf"] = tile8(hp["pb"])
    m["pm"] = hp["pm"]
    m["const_host"] = hp["const_host"]
    return m



def _pack(m):
    def packcols(entries, prows=128):
        cols = []
        off = {}
        n = 0
        for name, arr, r0, rows in entries:
            a = np.zeros((prows, arr.shape[1]))
            a[r0:r0 + rows] = arr[:rows]
            off[name] = n
            n += arr.shape[1]
            cols.append(a)
        return np.concatenate(cols, 1), off

    corrAmlo = np.zeros((128, 128))
    corrAmlo[112:128, 112:128] = m["corrAm"]
    negpWlo = np.zeros((128, 128))
    negpWlo[112:128, 112:128] = m["negpW"]
    negCN = np.zeros((128, 16))
    negCN[112:128] = -np.eye(16)
    om0pad = np.zeros((16, 128))
    om0pad[:, 0:16] = m["Om0"].T
    cid, offI = packcols([("ident32", np.eye(32), 0, 32)], prows=32)
    cT1a, off1a = packcols([
        ("W8c0", m["W8"][0:128], 0, 128),
        ("bdKg", m["bdKg"].T, 0, 128),
        ("W8c1mKg", np.vstack([m["W8c1mKg"], np.zeros((112, 16))]), 0, 16),
    ])
    cT1b, off1b = packcols([
        ("SF", m["SF"].T, 0, 128),
        ("G1", m["G1"].T, 0, 128),
    ])
    cT2a, off2a = packcols([
        ("bdAm", m["bdAm"].T, 0, 128),
        ("corrAmlo", corrAmlo.T[64:128], 64, 64),
    ])
    cT2b, off2b = packcols([
        ("SB", m["SB"].T, 0, 128),
        ("H1", m["H1"].T, 0, 128),
    ])
    cWW, offW = packcols([
        ("ww0", m["WEWA"][0:128], 0, 128),
    ])
    cT3a, off3a = packcols([
        ("bdpH", m["bdpH"].T, 0, 128),
        ("bdpWmSU", m["bdpWmSU"].T, 0, 128),
        ("negpWlo", negpWlo.T[64:128], 64, 64),
    ])
    cT3b, off3b = packcols([
        ("bdOmO", m["bdOmO"].T, 0, 128),
        ("bdOmT", m["bdOmT"].T, 0, 128),
    ])
    c16t, off16 = packcols([
        ("negCN16", negCN.T, 0, 16),
        ("Om0pad16", om0pad, 0, 16),
        ("ww1", m["WEWA"][128:144], 0, 16),
        ("ECF8", m["ECF"].T, 0, 16),
        ("pm16", m["pm"].reshape(16, 1), 0, 16),
        ("webias_r", m["webias"].reshape(1, 128), 0, 1),
        ("onesb", np.ones((1, 1)), 0, 1),
    ], prows=16)
    cFm = np.zeros((128, 10))
    cFm[0, 0] = m["const_host"]
    cFm[:, 1] = 1.0
    cFm[:, 2:3] = m["c0f"]
    cFm[:, 3:4] = m["abf"]
    cFm[:, 4:5] = m["phf"]
    cFm[:, 5:6] = m["pbf"]
    cFm[:, 6:7] = m["abf"]
    cFm[112:128, 6] = 0.0
    cFm[:, 7:8] = m["pbf"]
    cFm[112:128, 7] = 0.0
    cFm[:, 8:9] = m["c0f"]
    cFm[0:16, 8] = m["m8b"]
    cFm[:, 9] = m["webias"].reshape(128)
    offF = {"chost": 0, "ones": 1, "c0f": 2, "abf": 3, "phf": 4,
            "pbf": 5, "abf7z": 6, "pbf7z": 7, "mixbias": 8, "webias": 9}
    return {
        "cid": (cid.astype(BF16), offI),
        "cT1a": (cT1a.astype(BF16), off1a),
        "cT1b": (cT1b.astype(BF16), off1b),
        "cT2a": (cT2a.astype(BF16), off2a),
        "cT2b": (cT2b.astype(BF16), off2b),
        "cWW": (cWW.astype(BF16), offW),
        "cT3a": (cT3a.astype(BF16), off3a),
        "cT3b": (cT3b.astype(BF16), off3b),
        "c16t": (c16t.astype(BF16), off16),
        "cF": (cFm.astype(F32), offF),
    }


def _build_program(shapes, offs):
    import concourse.bacc as bacc
    import concourse.mybir as mybir
    from concourse import tile

    f32 = mybir.dt.float32
    bf16 = mybir.dt.bfloat16
    ALU = mybir.AluOpType
    ACT = mybir.ActivationFunctionType
    nc = bacc.Bacc("TRN2", target_bir_lowering=False, debug=False)
    obs_d = nc.declare_dram_parameter("obs", [32, 128], bf16, isOutput=False)
    cdr = {}
    for nm in ("cid", "cT1a", "cT1b", "cT2a", "cT2b", "cWW", "cT3a",
               "cT3b", "c16t"):
        rows = {"cid": 32, "c16t": 16}.get(nm, 128)
        cdr[nm] = nc.declare_dram_parameter(nm, [rows, shapes[nm][1]], bf16,
                                            isOutput=False)
    cF_d = nc.declare_dram_parameter("cF", list(shapes["cF"]), f32,
                                     isOutput=False)
    out_d = nc.declare_dram_parameter("out", [1, 1], f32, isOutput=True)

    (oI, o1a, o1b, o2a, o2b, oW, o3a, o3b, o16, oF) = (offs[n] for n in
        ("cid", "cT1a", "cT1b", "cT2a", "cT2b", "cWW", "cT3a", "cT3b",
         "c16t", "cF"))

    with tile.TileContext(nc) as tc:
        with (
            tc.tile_pool(name="const", bufs=1) as cpool,
            tc.tile_pool(name="sb", bufs=1) as sb,
            tc.tile_pool(name="ps", bufs=2, space="PSUM") as ps,
        ):
            tI = cpool.tile([32, shapes["cid"][1]], bf16)
            t1a = cpool.tile([128, shapes["cT1a"][1]], bf16)
            t1b = cpool.tile([128, shapes["cT1b"][1]], bf16)
            t2a = cpool.tile([128, shapes["cT2a"][1]], bf16)
            t2b = cpool.tile([128, shapes["cT2b"][1]], bf16)
            tWW = cpool.tile([128, shapes["cWW"][1]], bf16)
            t3a = cpool.tile([128, shapes["cT3a"][1]], bf16)
            t3b = cpool.tile([128, shapes["cT3b"][1]], bf16)
            t16 = cpool.tile([16, shapes["c16t"][1]], bf16)
            tF = cpool.tile([128, shapes["cF"][1]], f32)
            ob = sb.tile([32, 128], bf16)
            nc.sync.dma_start(ob[:], obs_d[:])
            nc.sync.dma_start(tI[:], cdr["cid"][:])
            nc.sync.dma_start(t1a[:], cdr["cT1a"][:])
            nc.scalar.dma_start(t1b[:], cdr["cT1b"][:])
            nc.gpsimd.dma_start(tF[:], cF_d[:])
            nc.gpsimd.dma_start(t16[:], cdr["c16t"][:])
            nc.sync.dma_start(t2a[:], cdr["cT2a"][:])
            nc.scalar.dma_start(t2b[:], cdr["cT2b"][:])
            nc.gpsimd.dma_start(tWW[:], cdr["cWW"][:])
            nc.sync.dma_start(t3a[:], cdr["cT3a"][:])
            nc.scalar.dma_start(t3b[:], cdr["cT3b"][:])

            # ---- Yf = transpose(ob) ----
            psY = ps.tile([128, 32], bf16, tag="pA")
            nc.tensor.transpose(psY[:], ob[:], tI[0:32, 0:32])
            Yf = sb.tile([128, 32], bf16)
            nc.vector.tensor_copy(Yf[:], psY[:])

            def K1(name, w=128):
                t, o = (t1a, o1a) if name in o1a else (t1b, o1b)
                return t[0:128, o[name]:o[name] + w]

            def K2(name, w=128):
                t, o = (t2a, o2a) if name in o2a else (t2b, o2b)
                return t[0:128, o[name]:o[name] + w]

            def K2lo(name, w=128):
                return t2a[64:128, o2a[name]:o2a[name] + w]

            def K3(name, w=128):
                t, o = (t3a, o3a) if name in o3a else (t3b, o3b)
                return t[0:128, o[name]:o[name] + w]

            def K3lo(name, w=128):
                return t3a[64:128, o3a[name]:o3a[name] + w]

            def K16(name, w=128):
                return t16[0:16, o16[name]:o16[name] + w]

            def KF(name):
                return tF[:, oF[name]:oF[name] + 1]

            # ---- psJ (j-grid, 31 cols); col0 block0 += W8 path ----
            psJ = ps.tile([128, 31], f32, tag="pB")
            nc.tensor.matmul(psJ[:], K1("bdKg"), Yf[:, 1:32],
                             start=True, stop=False)
            nc.tensor.matmul(psJ[0:16, 0:1], K1("W8c0", 16), Yf[:, 0:1],
                             start=False, stop=False)
            nc.tensor.matmul(psJ[0:16, 0:1],
                             t1a[0:16, o1a["W8c1mKg"]:o1a["W8c1mKg"] + 16],
                             Yf[0:16, 1:2], start=False, stop=True)
            X = sb.tile([128, 31], bf16)
            nc.vector.tensor_scalar_add(X[:, 1:31], psJ[:, 1:31], KF("c0f"))
            nc.vector.tensor_add(X[:, 0:1], psJ[:, 0:1], KF("mixbias"))

            # ---- forward conv scan (one accumulation group) ----
            psL = ps.tile([128, 31], f32, tag="pA")
            nc.tensor.matmul(psL[:], K1("SF"), X[:], start=True, stop=False)
            nc.tensor.matmul(psL[:, 1:31], K1("G1"), X[:, 0:30],
                             start=False, stop=True)
            Mf = sb.tile([128, 31], bf16)
            nc.vector.tensor_copy(Mf[:], psL[:])

            # ---- Ef[:,1:32] = bdAm @ Mf + abf (col31: corrAm, abf7z) ----
            Ef = sb.tile([128, 32], bf16)
            psA = ps.tile([128, 31], f32, tag="pB")
            nc.tensor.matmul(psA[:], K2("bdAm"), Mf[:], start=True,
                             stop=False)
            nc.tensor.matmul(psA[:, 30:31], K2lo("corrAmlo"),
                             Mf[64:128, 30:31], start=False, stop=True)
            nc.vector.tensor_scalar_add(Ef[:, 1:31], psA[:, 0:30], KF("abf"))
            nc.vector.tensor_add(Ef[:, 31:32], psA[:, 30:31], KF("abf7z"))

            # ---- backward conv scan ----
            psB = ps.tile([128, 31], f32, tag="pA")
            nc.tensor.matmul(psB[:], K2("SB"), Ef[:, 1:32],
                             start=True, stop=False)
            nc.tensor.matmul(psB[:, 0:30], K2("H1"), Ef[:, 2:32],
                             start=False, stop=True)
            nc.vector.tensor_copy(Ef[:, 1:32], psB[:])

            # ---- transient col 0: one accumulation group then copy ----
            psC = ps.tile([128, 1], f32, tag="pE")
            nc.tensor.matmul(psC[:], tWW[0:128, 0:128], Yf[:, 0:1],
                             start=True, stop=False)
            nc.tensor.matmul(psC[:], K16("ww1"), Yf[0:16, 1:2],
                             start=False, stop=False)
            nc.tensor.matmul(psC[:], K16("webias_r")[0:1, :],
                             K16("onesb", 1)[0:1, :], start=False, stop=False)
            nc.tensor.matmul(psC[:], K16("ECF8"), Ef[0:16, 1:2],
                             start=False, stop=True)
            nc.vector.tensor_copy(Ef[:, 0:1], psC[:])

            # ---- UOE ----
            UOE = sb.tile([128, 65], bf16)
            nc.gpsimd.memset(UOE[:, 64:65], 0.0)
            nc.vector.tensor_sub(UOE[0:16, 64:65], Ef[0:16, 0:1],
                                 K16("pm16", 1))
            psUO = ps.tile([128, 32], f32, tag="pB")
            nc.tensor.matmul(psUO[:], K3("bdpH"), Ef[:])
            nc.vector.scalar_tensor_tensor(UOE[:, 0:32], psUO[:], KF("phf"),
                                           Yf[:], op0=ALU.add,
                                           op1=ALU.subtract)
            psUE = ps.tile([128, 32], f32, tag="pA")
            nc.tensor.matmul(psUE[:], K3("bdpWmSU"), Ef[:],
                             start=True, stop=False)
            nc.tensor.matmul(psUE[:, 0:31], K16("negCN16"), Ef[0:16, 1:32],
                             start=False, stop=False)
            nc.tensor.matmul(psUE[:, 31:32], K3lo("negpWlo"),
                             Ef[64:128, 31:32], start=False, stop=True)
            nc.scalar.activation(UOE[:, 32:63], psUE[:, 0:31], ACT.Identity,
                                 bias=KF("pbf"))
            nc.vector.tensor_add(UOE[:, 63:64], psUE[:, 31:32],
                                  KF("pbf7z"))

            # ---- Z ; fused quad-reduce ----
            psZ = ps.tile([128, 65], f32, tag="pE")
            nc.tensor.matmul(psZ[:, 0:32], K3("bdOmO"), UOE[:, 0:32])
            nc.tensor.matmul(psZ[:, 32:64], K3("bdOmT"), UOE[:, 32:64])
            nc.tensor.matmul(psZ[:, 64:65], K16("Om0pad16"),
                             UOE[0:16, 64:65])
            P = sb.tile([128, 65], f32)
            acc = sb.tile([128, 1], f32)
            nc.vector.scalar_tensor_tensor(P[:], psZ[:], 1.0, UOE[:],
                                           op0=ALU.mult, op1=ALU.mult,
                                           accum_out=acc[:])
            psT = ps.tile([1, 1], f32, tag="pB")
            nc.tensor.matmul(psT[:], KF("ones"), acc[:],
                             start=True, stop=False)
            nc.tensor.matmul(psT[:], KF("chost"), KF("ones"),
                             start=False, stop=True)
            res = sb.tile([1, 1], f32)
            nc.vector.tensor_copy(res[:], psT[:])
            nc.sync.dma_start(out_d[:], res[:])

    nc.finalize()
    return nc


def _get_program(tabs):
    shapes = {k: v[0].shape for k, v in tabs.items()}
    offs = {k: v[1] for k, v in tabs.items()}
    key = tuple(sorted((k, s) for k, s in shapes.items()))
    if key not in _PROGRAM_CACHE:
        _PROGRAM_CACHE[key] = _build_program(shapes, offs)
    return _PROGRAM_CACHE[key]


def kernel(**inputs):
    from concourse.bass_utils import run_bass_kernel_spmd

    hp = host_prep_v5(inputs)
    m = build_mats_v5(hp)
    tabs = _pack(m)
    obs = np.asarray(inputs["observations"], F32).astype(BF16).reshape(32, 128)
    nc = _get_program(tabs)
    in_map = {"obs": obs}
    for k, (arr, _) in tabs.items():
        in_map[k] = arr
    res = run_bass_kernel_spmd(nc, [dict(in_map) for _ in range(8)],
                               list(range(8)))
    out = res.results[0]["out"]
    return np.asarray(out, dtype=np.float32).reshape(())


# revision 4
# speedup vs baseline: 1.0650x; 1.0367x over previous
"""Trainium2 Bass kernel for nn_LinearGaussianQ (folded-native bf16).

Reference: T=256-step linear-Gaussian smoother scan with a growing stack of
quadratic forms.  Reformulated (validated to ~2e-5 rel in f64):

  * parameter-only work (Kalman covariance pipeline, log-dets, trace series)
    is precomputed on host in f64 and folded into one constant, exactly like
    the reference precomputes its parameter inverses;
  * everything touching `observations` runs on device in bf16 (tolerance is
    2e-2; bf16 end-to-end error ~5e-5): the forward filter-mean recursion and
    the backward e-recursion become chunked matrix scans in a folded layout
    [128, C] with time t = 8c + l (partition-block l, column c) — one local
    block-triangular-Toeplitz [128x128] matmul plus a 1-chunk-band
    block-Toeplitz correction (state decay ||F^9|| ~ 2e-3, ||F^17|| ~ 3e-6
    makes the window truncation negligible);
  * the first TS=8 steps use exact time-varying maps composed on host
    (F_t/B_t converge to the steady-state maps at ~1e-6 by t=8);
  * all quadratic-form data terms are evaluated folded on all 128 partitions
    and reduced on-device to a single scalar.

obs is marshaled on host into the folded [128, 32] layout (pure
reshape/transpose/cast, no arithmetic) so it DMAs straight into place.

Sharding: the recursion is strictly sequential with tiny state (batch=1), so
the data-parallel hint degenerates; all 8 cores run the identical replicated
program and core 0's scalar is returned.
"""
import numpy as np
import ml_dtypes

T = 256
DZ = 16
TS = 8
LOG2PI = float(np.log(2.0 * np.pi))
F32 = np.float32
BF16 = ml_dtypes.bfloat16

_PROGRAM_CACHE = {}


def host_prep_v5(inputs):
    o = {k: np.asarray(v, np.float64) for k, v in inputs.items()}
    dz = DZ
    I = np.eye(dz)

    def cterm(dim, det):
        return -0.5 * (dim * LOG2PI + np.log(det))

    p_tr_prec = np.linalg.inv(o["p_trans_cov"])
    p_tr_det = np.linalg.det(o["p_trans_cov"])
    p_em_prec = np.linalg.inv(o["p_em_cov"])
    p_em_det = np.linalg.det(o["p_em_cov"])
    q_tr_prec = np.linalg.inv(o["q_trans_cov"])
    Om_obs = -0.5 * p_em_prec
    Om_tr = -0.5 * p_tr_prec
    Om0 = -0.5 * np.linalg.inv(o["p_prior_cov"])
    qW, qb = o["q_trans_w"], o["q_trans_b"]
    qC = o["q_trans_cov"]
    H, h, Rm = o["q_em_w"], o["q_em_b"], o["q_em_cov"]
    pW, pb = o["p_trans_w"], o["p_trans_b"]
    pH, ph = o["p_em_w"], o["p_em_b"]
    cm = qW.T @ q_tr_prec
    Phi = cm @ qW
    Cobs = pH.T @ Om_obs @ pH
    Ctr = -0.5 * pW.T @ p_tr_prec @ pW
    c1 = (cterm(dz, p_em_det) + cterm(dz, p_tr_det) + 0.5 * dz
          + 0.5 * dz * LOG2PI)

    def kgain(P_pred):
        S = H @ P_pred @ H.T + Rm
        Kg = P_pred @ H.T @ np.linalg.inv(S)
        return Kg, (I - Kg @ H) @ P_pred

    Kg0, P0 = kgain(o["q_prior_cov"])
    Pf = [P0]
    Kgs = [Kg0]
    Bs = [None]
    bcovs = [None]
    Ams = [None]
    abts = [None]
    for t in range(1, T):
        Pprev = Pf[-1]
        P_prec = np.linalg.inv(Pprev)
        bcov = np.linalg.inv(Phi + P_prec)
        Bs.append(bcov @ cm)
        bcovs.append(bcov)
        Ams.append(np.linalg.inv(I + Pprev @ Phi))
        abts.append(-(bcov @ cm @ qb))
        Kg, Pnew = kgain(qW @ Pprev @ qW.T + qC)
        Pf.append(Pnew)
        Kgs.append(Kg)

    const = cterm(dz, np.linalg.det(o["p_prior_cov"])) + cterm(dz, p_em_det)
    M = Om0.copy()
    for t in range(1, T):
        bcov = bcovs[t]
        const += np.trace((M + Cobs + Ctr) @ bcov)
        const += 0.5 * np.log(np.linalg.det(bcov)) + c1
        B = Bs[t]
        M = B.T @ (M + Cobs) @ B + (pW @ B - I).T @ Om_tr @ (pW @ B - I)
    const -= cterm(dz, np.linalg.det(Pf[-1]))

    P_ss = Pf[-1]
    tr = 0.0
    Rt = {T - 1: np.eye(dz)}
    for t in range(T - 2, TS - 1, -1):
        Rt[t] = Bs[t + 1] @ Rt[t + 1]
    for t in range(1, T):
        Rm1 = Rt.get(t - 1)
        Rcur = Rt.get(t)
        if Rm1 is None or Rcur is None:
            continue
        G = pH @ Rm1
        tr += np.einsum('ij,jl,lm,mi->', Om_obs, G, P_ss, G)
        Ae = pW @ Rm1 - Rcur
        tr += np.einsum('ij,jl,lm,mi->', Om_tr, Ae, P_ss, Ae)
    tr_p = np.trace(Om_obs @ pH @ P_ss @ pH)
    const_host = const + tr + tr_p + 0.5 * dz

    Fs = [None] + [(I - Kgs[t] @ H) @ qW for t in range(1, T)]
    c0s = [None] + [(I - Kgs[t] @ H) @ qb - Kgs[t] @ h for t in range(1, T)]

    A0 = Kg0
    b0 = (I - Kg0 @ H) @ o["q_prior_mean"] - Kg0 @ h
    Gp = [{0: A0}]
    ccp = [b0]
    for t in range(1, TS + 1):
        g = {s: Fs[t] @ mm for s, mm in Gp[t - 1].items()}
        g[t] = Kgs[t].copy()
        Gp.append(g)
        ccp.append(Fs[t] @ ccp[t - 1] + c0s[t])

    W8 = np.zeros((9 * 16, 16))
    for s, g in Gp[TS].items():
        W8[s * 16:(s + 1) * 16, :] = g.T
    m8b = ccp[TS]

    WA = np.zeros((9 * 16, 8 * 16))
    ab_bias = np.zeros(8 * 16)
    for t in range(1, TS + 1):
        for s, g in Gp[t - 1].items():
            AG = Ams[t] @ g
            WA[s * 16:(s + 1) * 16, (t - 1) * 16:t * 16] = AG.T
        ac = Ams[t] @ ccp[t - 1] + abts[t]
        ab_bias[(t - 1) * 16:t * 16] = ac

    Ec = {TS: I.copy()}
    Dc = {TS: {}}
    for s in range(TS - 1, -1, -1):
        Ec[s] = Bs[s + 1] @ Ec[s + 1]
        Dc[s] = {u: Bs[s + 1] @ g for u, g in Dc[s + 1].items()}
        Dc[s][s + 1] = I.copy()
    WE = np.zeros((8 * 16, 8 * 16))
    for s in range(TS):
        for u, g in Dc[s].items():
            WE[(u - 1) * 16:u * 16, s * 16:(s + 1) * 16] = g.T

    return dict(
        const_host=const_host,
        F=Fs[-1], B=Bs[-1], Kg=Kgs[-1], c0=c0s[-1],
        Am=Ams[-1], ab=abts[-1],
        pH=pH, ph=ph, pW=pW, pb=pb,
        Om_obs=Om_obs, Om_tr=Om_tr, Om0=Om0, pm=o["p_prior_mean"],
        W8=W8, m8b=m8b, WA=WA, ab_bias=ab_bias, WE=WE,
        Ec=[Ec[s] for s in range(TS)],
    )


def bd8(A):
    out = np.zeros((128, 128))
    for l in range(8):
        out[16 * l:16 * l + 16, 16 * l:16 * l + 16] = A
    return out


def tile8(v):
    return np.tile(np.asarray(v).reshape(-1), 8).reshape(128, 1)


def build_mats_v5(hp):
    F, B = hp["F"], hp["B"]
    m = {}
    Fp = [np.eye(DZ)]
    Bp = [np.eye(DZ)]
    for _ in range(26):
        Fp.append(Fp[-1] @ F)
        Bp.append(Bp[-1] @ B)
    SF = np.zeros((128, 128))
    SB = np.zeros((128, 128))
    for l in range(8):
        for i in range(8):
            if i <= l:
                SF[16 * l:16 * l + 16, 16 * i:16 * i + 16] = Fp[l - i]
            if i >= l:
                SB[16 * l:16 * l + 16, 16 * i:16 * i + 16] = Bp[i - l]
    m["SF"], m["SB"] = SF, SB
    for k in (1, 2):
        G = np.zeros((128, 128))
        Hk = np.zeros((128, 128))
        for l in range(8):
            for i in range(8):
                G[16 * l:16 * l + 16, 16 * i:16 * i + 16] = Fp[8 * k + l - i]
                Hk[16 * l:16 * l + 16, 16 * i:16 * i + 16] = Bp[8 * k + i - l]
        m[f"G{k}"] = G
        m[f"H{k}"] = Hk
    m["bdKg"] = bd8(hp["Kg"])
    m["bdAm"] = bd8(hp["Am"])
    m["bdpH"] = bd8(hp["pH"])
    SU = np.zeros((128, 128))
    for l in range(7):
        SU[16 * l:16 * l + 16, 16 * (l + 1):16 * (l + 1) + 16] = np.eye(16)
    m["bdpWmSU"] = bd8(hp["pW"]) - SU
    m["bdOmO"] = bd8(hp["Om_obs"])
    m["bdOmT"] = bd8(hp["Om_tr"])
    m["Om0"] = hp["Om0"]
    m["corrAm"] = np.eye(16) - hp["Am"]     # applied to block 7
    m["negpW"] = -hp["pW"]
    m["WEWA"] = hp["WA"] @ hp["WE"]          # [144, 128]
    m["webias"] = hp["WE"].T @ hp["ab_bias"]  # [128]
    m["W8"] = hp["W8"]
    m["W8c1mKg"] = hp["W8"][128:144] - hp["Kg"].T
    ECF = np.zeros((128, 16))
    for s in range(8):
        ECF[16 * s:16 * s + 16, :] = hp["Ec"][s]
    m["ECF"] = ECF
    m["m8b"] = hp["m8b"]
    m["c0f"] = tile8(hp["c0"])
    m["abf"] = tile8(hp["ab"])
    m["phf"] = tile8(hp["ph"])
    m["pbf"] = tile8(hp["pb"])
    m["pm"] = hp["pm"]
    m["const_host"] = hp["const_host"]
    return m



def _pack(m):
    def packcols(entries, prows=128):
        cols = []
        off = {}
        n = 0
        for name, arr, r0, rows in entries:
            a = np.zeros((prows, arr.shape[1]))
            a[r0:r0 + rows] = arr[:rows]
            off[name] = n
            n += arr.shape[1]
            cols.append(a)
        return np.concatenate(cols, 1), off

    corrAmlo = np.zeros((128, 128))
    corrAmlo[112:128, 112:128] = m["corrAm"]
    negpWlo = np.zeros((128, 128))
    negpWlo[112:128, 112:128] = m["negpW"]
    negCN = np.zeros((128, 16))
    negCN[112:128] = -np.eye(16)
    om0pad = np.zeros((16, 128))
    om0pad[:, 0:16] = m["Om0"].T
    cT1a, off1a = packcols([
        ("W8c0", m["W8"][0:128], 0, 128),
        ("bdKg", m["bdKg"].T, 0, 128),
        ("W8c1mKg", np.vstack([m["W8c1mKg"], np.zeros((112, 16))]), 0, 16),
    ])
    cT1b, off1b = packcols([
        ("SF", m["SF"].T, 0, 128),
        ("G1", m["G1"].T, 0, 128),
    ])
    cT2a, off2a = packcols([
        ("bdAm", m["bdAm"].T, 0, 128),
        ("corrAmlo", corrAmlo.T[64:128], 64, 64),
    ])
    cT2b, off2b = packcols([
        ("SB", m["SB"].T, 0, 128),
        ("H1", m["H1"].T, 0, 128),
    ])
    cWW, offW = packcols([
        ("ww0", m["WEWA"][0:128], 0, 128),
    ])
    cT3a, off3a = packcols([
        ("bdpH", m["bdpH"].T, 0, 128),
        ("bdpWmSU", m["bdpWmSU"].T, 0, 128),
        ("negpWlo", negpWlo.T[64:128], 64, 64),
    ])
    cT3b, off3b = packcols([
        ("bdOmO", m["bdOmO"].T, 0, 128),
        ("bdOmT", m["bdOmT"].T, 0, 128),
    ])
    c16t, off16 = packcols([
        ("negCN16", negCN.T, 0, 16),
        ("Om0pad16", om0pad, 0, 16),
        ("ww1", m["WEWA"][128:144], 0, 16),
        ("ECF8", m["ECF"].T, 0, 16),
        ("pm16", m["pm"].reshape(16, 1), 0, 16),
        ("webias_r", m["webias"].reshape(1, 128), 0, 1),
        ("onesb", np.ones((1, 1)), 0, 1),
    ], prows=16)
    cFm = np.zeros((128, 10))
    cFm[0, 0] = m["const_host"]
    cFm[:, 1] = 1.0
    cFm[:, 2:3] = m["c0f"]
    cFm[:, 3:4] = m["abf"]
    cFm[:, 4:5] = m["phf"]
    cFm[:, 5:6] = m["pbf"]
    cFm[:, 6:7] = m["abf"]
    cFm[112:128, 6] = 0.0
    cFm[:, 7:8] = m["pbf"]
    cFm[112:128, 7] = 0.0
    cFm[:, 8:9] = m["c0f"]
    cFm[0:16, 8] = m["m8b"]
    cFm[:, 9] = m["webias"].reshape(128)
    offF = {"chost": 0, "ones": 1, "c0f": 2, "abf": 3, "phf": 4,
            "pbf": 5, "abf7z": 6, "pbf7z": 7, "mixbias": 8, "webias": 9}
    return {
        "cT1a": (cT1a.astype(BF16), off1a),
        "cT1b": (cT1b.astype(BF16), off1b),
        "cT2a": (cT2a.astype(BF16), off2a),
        "cT2b": (cT2b.astype(BF16), off2b),
        "cWW": (cWW.astype(BF16), offW),
        "cT3a": (cT3a.astype(BF16), off3a),
        "cT3b": (cT3b.astype(BF16), off3b),
        "c16t": (c16t.astype(BF16), off16),
        "cF": (cFm.astype(F32), offF),
    }


def _build_program(shapes, offs):
    import concourse.bacc as bacc
    import concourse.mybir as mybir
    from concourse import tile

    f32 = mybir.dt.float32
    bf16 = mybir.dt.bfloat16
    ALU = mybir.AluOpType
    ACT = mybir.ActivationFunctionType
    nc = bacc.Bacc("TRN2", target_bir_lowering=False, debug=False)
    obs_d = nc.declare_dram_parameter("obs", [128, 32], bf16, isOutput=False)
    cdr = {}
    for nm in ("cT1a", "cT1b", "cT2a", "cT2b", "cWW", "cT3a",
               "cT3b", "c16t"):
        rows = {"c16t": 16}.get(nm, 128)
        cdr[nm] = nc.declare_dram_parameter(nm, [rows, shapes[nm][1]], bf16,
                                            isOutput=False)
    cF_d = nc.declare_dram_parameter("cF", list(shapes["cF"]), f32,
                                     isOutput=False)
    out_d = nc.declare_dram_parameter("out", [1, 1], f32, isOutput=True)

    (o1a, o1b, o2a, o2b, oW, o3a, o3b, o16, oF) = (offs[n] for n in
        ("cT1a", "cT1b", "cT2a", "cT2b", "cWW", "cT3a", "cT3b",
         "c16t", "cF"))

    with tile.TileContext(nc) as tc:
        with (
            tc.tile_pool(name="const", bufs=1) as cpool,
            tc.tile_pool(name="sb", bufs=1) as sb,
            tc.tile_pool(name="ps", bufs=2, space="PSUM") as ps,
        ):
            t1a = cpool.tile([128, shapes["cT1a"][1]], bf16)
            t1b = cpool.tile([128, shapes["cT1b"][1]], bf16)
            t2a = cpool.tile([128, shapes["cT2a"][1]], bf16)
            t2b = cpool.tile([128, shapes["cT2b"][1]], bf16)
            tWW = cpool.tile([128, shapes["cWW"][1]], bf16)
            t3a = cpool.tile([128, shapes["cT3a"][1]], bf16)
            t3b = cpool.tile([128, shapes["cT3b"][1]], bf16)
            t16 = cpool.tile([16, shapes["c16t"][1]], bf16)
            tF = cpool.tile([128, shapes["cF"][1]], f32)
            Yf = sb.tile([128, 32], bf16)
            nc.sync.dma_start(Yf[:], obs_d[:])
            nc.sync.dma_start(t1a[:], cdr["cT1a"][:])
            nc.scalar.dma_start(t1b[:], cdr["cT1b"][:])
            nc.gpsimd.dma_start(tF[:], cF_d[:])
            nc.gpsimd.dma_start(t16[:], cdr["c16t"][:])
            nc.sync.dma_start(t2a[:], cdr["cT2a"][:])
            nc.scalar.dma_start(t2b[:], cdr["cT2b"][:])
            nc.gpsimd.dma_start(tWW[:], cdr["cWW"][:])
            nc.sync.dma_start(t3a[:], cdr["cT3a"][:])
            nc.scalar.dma_start(t3b[:], cdr["cT3b"][:])

            def K1(name, w=128):
                t, o = (t1a, o1a) if name in o1a else (t1b, o1b)
                return t[0:128, o[name]:o[name] + w]

            def K2(name, w=128):
                t, o = (t2a, o2a) if name in o2a else (t2b, o2b)
                return t[0:128, o[name]:o[name] + w]

            def K2lo(name, w=128):
                return t2a[64:128, o2a[name]:o2a[name] + w]

            def K3(name, w=128):
                t, o = (t3a, o3a) if name in o3a else (t3b, o3b)
                return t[0:128, o[name]:o[name] + w]

            def K3lo(name, w=128):
                return t3a[64:128, o3a[name]:o3a[name] + w]

            def K16(name, w=128):
                return t16[0:16, o16[name]:o16[name] + w]

            def KF(name):
                return tF[:, oF[name]:oF[name] + 1]

            # ---- psJ (j-grid, 31 cols); col0 block0 += W8 path ----
            psJ = ps.tile([128, 31], f32, tag="pB")
            nc.tensor.matmul(psJ[:], K1("bdKg"), Yf[:, 1:32],
                             start=True, stop=False)
            nc.tensor.matmul(psJ[0:16, 0:1], K1("W8c0", 16), Yf[:, 0:1],
                             start=False, stop=False)
            nc.tensor.matmul(psJ[0:16, 0:1],
                             t1a[0:16, o1a["W8c1mKg"]:o1a["W8c1mKg"] + 16],
                             Yf[0:16, 1:2], start=False, stop=True)
            X = sb.tile([128, 31], bf16)
            nc.vector.tensor_scalar_add(X[:, 1:31], psJ[:, 1:31], KF("c0f"))
            nc.vector.tensor_add(X[:, 0:1], psJ[:, 0:1], KF("mixbias"))

            # ---- forward conv scan (one accumulation group) ----
            psL = ps.tile([128, 31], f32, tag="pA")
            nc.tensor.matmul(psL[:], K1("SF"), X[:], start=True, stop=False)
            nc.tensor.matmul(psL[:, 1:31], K1("G1"), X[:, 0:30],
                             start=False, stop=True)
            Mf = sb.tile([128, 31], bf16)
            nc.vector.tensor_copy(Mf[:], psL[:])

            # ---- Ef[:,1:32] = bdAm @ Mf + abf (col31: corrAm, abf7z) ----
            Ef = sb.tile([128, 32], bf16)
            psA = ps.tile([128, 31], f32, tag="pB")
            nc.tensor.matmul(psA[:], K2("bdAm"), Mf[:], start=True,
                             stop=False)
            nc.tensor.matmul(psA[:, 30:31], K2lo("corrAmlo"),
                             Mf[64:128, 30:31], start=False, stop=True)
            nc.vector.tensor_scalar_add(Ef[:, 1:31], psA[:, 0:30], KF("abf"))
            nc.vector.tensor_add(Ef[:, 31:32], psA[:, 30:31], KF("abf7z"))

            # ---- backward conv scan ----
            psB = ps.tile([128, 31], f32, tag="pA")
            nc.tensor.matmul(psB[:], K2("SB"), Ef[:, 1:32],
                             start=True, stop=False)
            nc.tensor.matmul(psB[:, 0:30], K2("H1"), Ef[:, 2:32],
                             start=False, stop=True)
            nc.vector.tensor_copy(Ef[:, 1:32], psB[:])

            # ---- transient col 0: one accumulation group then copy ----
            psC = ps.tile([128, 1], f32, tag="pE")
            nc.tensor.matmul(psC[:], tWW[0:128, 0:128], Yf[:, 0:1],
                             start=True, stop=False)
            nc.tensor.matmul(psC[:], K16("ww1"), Yf[0:16, 1:2],
                             start=False, stop=False)
            nc.tensor.matmul(psC[:], K16("webias_r")[0:1, :],
                             K16("onesb", 1)[0:1, :], start=False, stop=False)
            nc.tensor.matmul(psC[:], K16("ECF8"), Ef[0:16, 1:2],
                             start=False, stop=True)
            nc.vector.tensor_copy(Ef[:, 0:1], psC[:])

            # ---- UOE ----
            UOE = sb.tile([128, 65], bf16)
            nc.gpsimd.memset(UOE[:, 64:65], 0.0)
            nc.vector.tensor_sub(UOE[0:16, 64:65], Ef[0:16, 0:1],
                                 K16("pm16", 1))
            psUO = ps.tile([128, 32], f32, tag="pB")
            nc.tensor.matmul(psUO[:], K3("bdpH"), Ef[:])
            nc.vector.scalar_tensor_tensor(UOE[:, 0:32], psUO[:], KF("phf"),
                                           Yf[:], op0=ALU.add,
                                           op1=ALU.subtract)
            psUE = ps.tile([128, 32], f32, tag="pA")
            nc.tensor.matmul(psUE[:], K3("bdpWmSU"), Ef[:],
                             start=True, stop=False)
            nc.tensor.matmul(psUE[:, 0:31], K16("negCN16"), Ef[0:16, 1:32],
                             start=False, stop=False)
            nc.tensor.matmul(psUE[:, 31:32], K3lo("negpWlo"),
                             Ef[64:128, 31:32], start=False, stop=True)
            nc.scalar.activation(UOE[:, 32:63], psUE[:, 0:31], ACT.Identity,
                                 bias=KF("pbf"))
            nc.vector.tensor_add(UOE[:, 63:64], psUE[:, 31:32],
                                  KF("pbf7z"))

            # ---- Z ; fused quad-reduce ----
            psZ = ps.tile([128, 65], f32, tag="pE")
            nc.tensor.matmul(psZ[:, 0:32], K3("bdOmO"), UOE[:, 0:32])
            nc.tensor.matmul(psZ[:, 32:64], K3("bdOmT"), UOE[:, 32:64])
            nc.tensor.matmul(psZ[:, 64:65], K16("Om0pad16"),
                             UOE[0:16, 64:65])
            P = sb.tile([128, 65], f32)
            acc = sb.tile([128, 1], f32)
            nc.vector.scalar_tensor_tensor(P[:], psZ[:], 1.0, UOE[:],
                                           op0=ALU.mult, op1=ALU.mult,
                                           accum_out=acc[:])
            psT = ps.tile([1, 1], f32, tag="pB")
            nc.tensor.matmul(psT[:], KF("ones"), acc[:],
                             start=True, stop=False)
            nc.tensor.matmul(psT[:], KF("chost"), KF("ones"),
                             start=False, stop=True)
            res = sb.tile([1, 1], f32)
            nc.vector.tensor_copy(res[:], psT[:])
            nc.sync.dma_start(out_d[:], res[:])

    nc.finalize()
    return nc


def _get_program(tabs):
    shapes = {k: v[0].shape for k, v in tabs.items()}
    offs = {k: v[1] for k, v in tabs.items()}
    key = tuple(sorted((k, s) for k, s in shapes.items()))
    if key not in _PROGRAM_CACHE:
        _PROGRAM_CACHE[key] = _build_program(shapes, offs)
    return _PROGRAM_CACHE[key]


def kernel(**inputs):
    from concourse.bass_utils import run_bass_kernel_spmd

    hp = host_prep_v5(inputs)
    m = build_mats_v5(hp)
    tabs = _pack(m)
    obs = np.ascontiguousarray(
        np.asarray(inputs["observations"], F32).astype(BF16)
        .reshape(32, 128).T)
    nc = _get_program(tabs)
    in_map = {"obs": obs}
    for k, (arr, _) in tabs.items():
        in_map[k] = arr
    res = run_bass_kernel_spmd(nc, [dict(in_map) for _ in range(8)],
                               list(range(8)))
    out = res.results[0]["out"]
    return np.asarray(out, dtype=np.float32).reshape(())
